# revision 31
# baseline (speedup 1.0000x reference)
"""Complex multi-head attention (B=4, S=2048, D=512, H=8) on 8 TRN2 NeuronCores.

Sharding: core c handles batch b = c//2 and head group hg = c%2 (4 heads each).
Weights are head-sliced host-side; each core computes its 4 heads' attention and
a partial output projection; the host sums the two partials per batch.

All complex arithmetic is expanded host-side into real concatenated operands so
that on-device every step is a plain matmul accumulation (bf16 inputs with f32
PSUM accumulation by default, see BF16_MODE):
  - x is passed as XT = [x_b.T.re ; x_b.T.im]  (contraction dim 1024, on SBUF
    partitions in 8 chunks of 128)
  - Q^T/K^T are produced per head in [64 d_re | 64 d_im] x 2048 layout, so the
    Hermitian score Re(conj(q)k) is a single K=128 matmul.
  - softmax is computed without max subtraction (|scores| <= ~18 on this
    distribution, exp stays well inside fp32 range); the denominator comes from
    an all-ones stationary matmul over Vector-engine pair-sums of P^T chunks.
"""

import os

import numpy as np

import concourse.mybir as mybir
import concourse.tile as tile
from concourse import bacc
from concourse.bass import ds, ts
from concourse.bass_utils import run_bass_kernel_spmd

F32 = mybir.dt.float32
F32R = mybir.dt.float32r
BF16 = mybir.dt.bfloat16
# 0: all fp32r; 1: bf16 projection inputs (halves the DMA-bound startup ramp);
# 2: bf16 everywhere (bf16 matmuls avoid the fp32r self-loading weight
# penalty, ~216 vs ~260 ns per matmul). Measured rel err: 6e-4 / 7e-3 / 9e-3
# against a 2e-2 budget.
BF16_MODE = int(os.environ.get("KERNEL_BF16", "2"))
BF16_PROJ = BF16_MODE >= 1
PROJ_DT = BF16 if BF16_PROJ else F32R
ATTN_DT = BF16 if BF16_MODE >= 2 else F32R

B, S, D = 4, 2048, 512
H, Dh = 8, 64
HPC = 4          # heads per core
SCALE = 1.0 / 8.0  # 1/sqrt(Dh)

_NC = None


def _build():
    nc = bacc.Bacc("TRN2", target_bir_lowering=False, debug=False, num_devices=8)

    xt_d = nc.declare_dram_parameter("xt", [128, 8, S], PROJ_DT, isOutput=False)
    wq_d = nc.declare_dram_parameter("wq", [128, 8, HPC, 128], PROJ_DT, isOutput=False)
    wk_d = nc.declare_dram_parameter("wk", [128, 8, HPC, 128], PROJ_DT, isOutput=False)
    wv_d = nc.declare_dram_parameter("wv", [128, 8, 512], PROJ_DT, isOutput=False)
    r_d = nc.declare_dram_parameter("r", [128, HPC, 1024], ATTN_DT, isOutput=False)
    ones_d = nc.declare_dram_parameter("ones", [128, 128], ATTN_DT, isOutput=False)
    out_d = nc.declare_dram_parameter("out", [S, 1024], F32, isOutput=True)

    Exp = mybir.ActivationFunctionType.Exp

    with tile.TileContext(nc) as tc:
        with tc.tile_pool(name="consts", bufs=1) as consts:
            ones = consts.tile([128, 128], ATTN_DT)
            nc.sync.dma_start(out=ones[:, :], in_=ones_d[:, :])

            with tc.tile_pool(name="qkv", bufs=1) as qkv:
                qt = qkv.tile([128, HPC, S], ATTN_DT)   # [d_ri, h, q]
                kt = qkv.tile([128, HPC, S], ATTN_DT)   # [d_ri, h, k]
                v = qkv.tile([128, 16, 512], ATTN_DT)  # [k%128, k//128, h*128+ri*64+d]

                # ---- phase 1: projections ----
                with (
                    tc.tile_pool(name="ph1", bufs=1) as ph1,
                    tc.tile_pool(name="p1ps", bufs=1, space="PSUM") as p1ps,
                ):
                    xt_s = ph1.tile([128, 8, S], PROJ_DT)
                    w_s = ph1.tile([128, 8, HPC, 128], PROJ_DT)  # wq, then wk
                    wv_s = ph1.tile([128, 8, 512], PROJ_DT)
                    for c in range(8):
                        nc.sync.dma_start(
                            out=xt_s[:, c, 0:1024], in_=xt_d[:, c, 0:1024]
                        )
                        nc.sync.dma_start(
                            out=xt_s[:, c, 1024:2048], in_=xt_d[:, c, 1024:2048]
                        )
                        nc.gpsimd.dma_start(out=w_s[:, c, :, :], in_=wq_d[:, c, :, :])
                    nc.gpsimd.dma_start(out=wv_s[:, :, :], in_=wv_d[:, :, :])

                    # c-major waves of 8 concurrent PSUM accumulation chains:
                    # during the initial DMA, every arriving xt chunk feeds 8
                    # matmuls instead of blocking one group-at-a-time.
                    def _qk_wave(dst, heads, group_major=False):
                        groups = [(h, tg) for h in heads for tg in range(4)]
                        if group_major:
                            # all data resident: drain each group right after
                            # its accumulation so copies don't bunch at the end
                            for gi, (h, tg) in enumerate(groups):
                                gt = p1ps.tile(
                                    [128, 512], F32, name=f"acc{gi}", tag=f"acc{gi}"
                                )
                                for c in range(8):
                                    nc.tensor.matmul(
                                        gt[:, :],
                                        lhsT=w_s[:, c, h, :],
                                        rhs=xt_s[:, c, ts(tg, 512)],
                                        start=(c == 0),
                                        stop=(c == 7),
                                    )
                                nc.vector.tensor_copy(
                                    out=dst[:, h, ts(tg, 512)], in_=gt[:, :]
                                )
                            return
                        tiles = [
                            p1ps.tile([128, 512], F32, name=f"acc{gi}", tag=f"acc{gi}")
                            for gi in range(len(groups))
                        ]
                        for c in range(8):
                            for gi, (h, tg) in enumerate(groups):
                                nc.tensor.matmul(
                                    tiles[gi][:, :],
                                    lhsT=w_s[:, c, h, :],
                                    rhs=xt_s[:, c, ts(tg, 512)],
                                    start=(c == 0),
                                    stop=(c == 7),
                                )
                        for gi, (h, tg) in enumerate(groups):
                            nc.vector.tensor_copy(
                                out=dst[:, h, ts(tg, 512)], in_=tiles[gi][:, :]
                            )

                    def _v_wave(tbs):
                        tbs = list(tbs)
                        tiles = [
                            p1ps.tile([128, 512], F32, name=f"acc{gi}", tag=f"acc{gi}")
                            for gi in range(len(list(tbs)))
                        ]
                        for c in range(8):
                            for gi, tb in enumerate(tbs):
                                nc.tensor.matmul(
                                    tiles[gi][:, :],
                                    lhsT=xt_s[:, c, ts(tb, 128)],
                                    rhs=wv_s[:, c, :],
                                    start=(c == 0),
                                    stop=(c == 7),
                                )
                        for gi, tb in enumerate(tbs):
                            nc.scalar.copy(out=v[:, tb, :], in_=tiles[gi][:, :])

                    # Q^T per head: [128 = (64 re | 64 im), 2048 tokens]
                    _qk_wave(qt, (0, 1))
                    _qk_wave(qt, (2, 3))
                    # wk overwrites the wq slot; the WAR wait on Q's last reads
                    # is hidden behind the V projection below
                    for c in range(8):
                        nc.gpsimd.dma_start(out=w_s[:, c, :, :], in_=wk_d[:, c, :, :])
                    # V: [token, 4h x (64 re | 64 im)] in 16 chunks of 128 tokens
                    _v_wave(range(8))
                    _v_wave(range(8, 16))
                    # K^T per head
                    _qk_wave(kt, (0, 1))
                    _qk_wave(kt, (2, 3), group_major=True)

                # ---- phases 2+3 ----
                with tc.tile_pool(name="p23", bufs=1) as p23:
                    ot = p23.tile([128, HPC, S], ATTN_DT)  # [(64 re|64 im), h, q]
                    r_s = p23.tile([128, HPC, 1024], ATTN_DT)
                    nc.gpsimd.dma_start(out=r_s[:, :, :], in_=r_d[:, :, :])

                    # ---- phase 2: attention ----
                    with (
                        tc.tile_pool(name="st", bufs=2, space="PSUM") as stp,
                        tc.tile_pool(name="ov", bufs=1, space="PSUM") as ovp,
                        tc.tile_pool(name="pt", bufs=4) as ptp,
                        tc.tile_pool(name="pair", bufs=3) as prp,
                        tc.tile_pool(name="misc", bufs=1) as miscp,
                    ):
                        for h in range(HPC):
                            for qh in range(2):
                                o_ps0 = ovp.tile([128, 512], F32, tag="o0")
                                o_ps1 = ovp.tile([128, 512], F32, tag="o1")
                                o_halves = (o_ps0, o_ps1)
                                d_ps = ovp.tile([128, 1024], F32, tag="d")
                                pts = []
                                pairs = []

                                def _emit_dmm(pi, g, stop=False):
                                    nc.tensor.matmul(
                                        d_ps[:, ts(g, 512)],
                                        lhsT=ones[:, :],
                                        rhs=pairs[pi][:, ts(g, 512)],
                                        start=(pi == 0),
                                        stop=stop,
                                    )

                                def _emit_scores(kc):
                                    st = stp.tile([128, 1024], F32)
                                    for g in range(2):
                                        qoff = qh * 1024 + g * 512
                                        nc.tensor.matmul(
                                            st[:, ts(g, 512)],
                                            lhsT=kt[:, h, ts(kc, 128)],
                                            rhs=qt[:, h, ds(qoff, 512)],
                                            start=True,
                                            stop=True,
                                        )
                                    return st

                                sts = {0: _emit_scores(0)}
                                for kc in range(16):
                                    pt_t = ptp.tile([128, 1024], ATTN_DT)
                                    nc.scalar.activation(
                                        out=pt_t[:, :], in_=sts.pop(kc)[:, :],
                                        func=Exp, scale=SCALE,
                                    )
                                    pts.append(pt_t)
                                    # scores for the NEXT chunk go on the PE
                                    # queue ahead of this chunk's exp-dependent
                                    # matmuls, so the PE computes them while
                                    # the ScalarE runs exp (keeps ACT streaming)
                                    if kc + 1 < 16:
                                        sts[kc + 1] = _emit_scores(kc + 1)
                                    for g in range(2):
                                        nc.tensor.matmul(
                                            o_halves[g][:, :],
                                            lhsT=v[:, kc, ds(h * 128, 128)],
                                            rhs=pt_t[:, ts(g, 512)],
                                            start=(kc == 0),
                                            stop=(kc == 15),
                                        )
                                    if kc % 2 == 1 and kc < 14:
                                        # denominator: pair-sum P^T chunks on
                                        # the Vector engine, then one all-ones
                                        # matmul per pair (instead of two)
                                        pr = prp.tile([128, 1024], ATTN_DT)
                                        nc.vector.tensor_add(
                                            pr[:, :], pts[kc - 1][:, :], pts[kc][:, :]
                                        )
                                        pairs.append(pr)
                                        # the PREVIOUS pair's denominator
                                        # matmuls: by now its vector add is done
                                        if len(pairs) >= 2:
                                            for g in range(2):
                                                _emit_dmm(len(pairs) - 2, g)
                                # tail: last pair, then the final two chunks go
                                # straight into the denominator (no pair-add on
                                # the critical path after the last exp)
                                for g in range(2):
                                    _emit_dmm(6, g)
                                for kc_t in (14, 15):
                                    for g in range(2):
                                        nc.tensor.matmul(
                                            d_ps[:, ts(g, 512)],
                                            lhsT=ones[:, :],
                                            rhs=pts[kc_t][:, ts(g, 512)],
                                            start=False,
                                            stop=(kc_t == 15),
                                        )
                                recip = miscp.tile([128, 1024], F32, tag="recip")
                                nc.vector.reciprocal_approx_fast(
                                    out=recip[:, :], in_=d_ps[:, :]
                                )
                                for g in range(2):
                                    nc.vector.tensor_mul(
                                        ot[:, h, ds(qh * 1024 + g * 512, 512)],
                                        o_halves[g][:, :],
                                        recip[:, ts(g, 512)],
                                    )

                    # ---- phase 3: output projection (partial over this core's heads) ----
                    with (
                        tc.tile_pool(name="yps", bufs=2, space="PSUM") as yps,
                        tc.tile_pool(name="ysb", bufs=4) as ysb,
                    ):
                        for tb in range(16):
                            y_ps = yps.tile([128, 1024], F32)
                            for g in range(2):
                                for hc in range(HPC):
                                    nc.tensor.matmul(
                                        y_ps[:, ts(g, 512)],
                                        lhsT=ot[:, hc, ts(tb, 128)],
                                        rhs=r_s[:, hc, ts(g, 512)],
                                        start=(hc == 0),
                                        stop=(hc == 3),
                                    )
                            y_s = ysb.tile([128, 1024], F32)
                            if tb % 2 == 0:
                                nc.vector.tensor_copy(out=y_s[:, :], in_=y_ps[:, :])
                                nc.sync.dma_start(
                                    out=out_d[ts(tb, 128), :], in_=y_s[:, :]
                                )
                            else:
                                nc.scalar.copy(out=y_s[:, :], in_=y_ps[:, :])
                                nc.gpsimd.dma_start(
                                    out=out_d[ts(tb, 128), :], in_=y_s[:, :]
                                )

    nc.compile()
    return nc


def _wcat_head(w_h):
    """[64, 512] complex head-slice of a projection weight -> [1024, 128] real
    stationary block: out column j<64 produces re(head feature j), j>=64 im."""
    wr = np.ascontiguousarray(w_h.real).astype(np.float32)
    wi = np.ascontiguousarray(w_h.imag).astype(np.float32)
    top = np.concatenate([wr.T, wi.T], axis=1)     # x_re rows
    bot = np.concatenate([-wi.T, wr.T], axis=1)    # x_im rows
    return np.concatenate([top, bot], axis=0)      # [1024, 128]


def _core_inputs(x, wq, wk, wv, wo, core):
    b, hg = divmod(core, 2)
    heads = [hg * HPC + h for h in range(HPC)]

    xt = np.concatenate(
        [x[b].T.real.astype(np.float32), x[b].T.imag.astype(np.float32)], axis=0
    )  # [1024, 2048]
    xt = np.ascontiguousarray(xt.reshape(8, 128, S).transpose(1, 0, 2))

    def _wqk(w):
        blocks = np.stack(
            [_wcat_head(w[gh * Dh : (gh + 1) * Dh]) for gh in heads]
        )  # [4, 1024, 128]
        return np.ascontiguousarray(
            blocks.reshape(HPC, 8, 128, 128).transpose(2, 1, 0, 3)
        )  # [128, 8, 4, 128]

    wv_cat = np.concatenate(
        [_wcat_head(wv[gh * Dh : (gh + 1) * Dh]) for gh in heads], axis=1
    )  # [1024, 512]
    wv_cat = np.ascontiguousarray(wv_cat.reshape(8, 128, 512).transpose(1, 0, 2))

    r_blocks = []
    for gh in heads:
        wo_h = wo[:, gh * Dh : (gh + 1) * Dh]  # [512, 64] complex
        wor = np.ascontiguousarray(wo_h.real).astype(np.float32)
        woi = np.ascontiguousarray(wo_h.imag).astype(np.float32)
        top = np.concatenate([wor.T, woi.T], axis=1)    # O_re rows -> [64, 1024]
        bot = np.concatenate([-woi.T, wor.T], axis=1)   # O_im rows
        r_blocks.append(np.concatenate([top, bot], axis=0))  # [128, 1024]
    r_cat = np.concatenate(r_blocks, axis=0)  # [512, 1024]
    r_cat = np.ascontiguousarray(r_cat.reshape(HPC, 128, 1024).transpose(1, 0, 2))

    out = {
        "xt": xt,
        "wq": _wqk(wq),
        "wk": _wqk(wk),
        "wv": wv_cat,
        "r": r_cat,
        "ones": np.ones((128, 128), dtype=np.float32),
    }
    if BF16_PROJ:
        import ml_dtypes

        for k in ("xt", "wq", "wk", "wv"):
            out[k] = out[k].astype(ml_dtypes.bfloat16)
    if BF16_MODE >= 2:
        import ml_dtypes

        for k in ("r", "ones"):
            out[k] = out[k].astype(ml_dtypes.bfloat16)
    return out


def kernel(x, wq, wk, wv, wo):
    global _NC
    x = np.asarray(x)
    wq = np.asarray(wq)
    wk = np.asarray(wk)
    wv = np.asarray(wv)
    wo = np.asarray(wo)

    if _NC is None:
        _NC = _build()

    in_maps = [_core_inputs(x, wq, wk, wv, wo, c) for c in range(8)]

    trace = os.environ.get("KERNEL_PROFILE", "0") == "1"
    kwargs = {}
    if trace:
        _install_profile_shim()
        kwargs = {"trace": True}
    res = run_bass_kernel_spmd(_NC, in_maps, core_ids=list(range(8)), **kwargs)
    if trace:
        print(f"HW exec time: {res.exec_time_ns} ns")

    out = np.zeros((B, S, D), dtype=np.complex64)
    for c in range(8):
        b = c // 2
        y = res.results[c]["out"]
        out[b] += y[:, :512] + 1j * y[:, 512:]
    return out


def _install_profile_shim():
    """Register the NTFF profile hook for axon (missing antenv.axon_hooks)."""
    import contextlib
    import ctypes
    import sys
    import types

    try:
        import antenv.axon_hooks  # noqa: F401

        return
    except ImportError:
        pass

    so_path = "/opt/axon/libaxon_pjrt.so"
    lib = ctypes.CDLL(so_path)
    if not hasattr(lib, "axon_start_nrt_profile"):
        return
    lib.axon_start_nrt_profile.argtypes = [
        ctypes.POINTER(ctypes.c_int64),
        ctypes.c_size_t,
    ]
    lib.axon_start_nrt_profile.restype = ctypes.c_int64
    lib.axon_stop_nrt_profile.argtypes = [ctypes.c_char_p]
    lib.axon_stop_nrt_profile.restype = ctypes.c_int64

    @contextlib.contextmanager
    def _hook(output_dir, device_ids):
        import jax

        jax.devices()
        if device_ids:
            ids = (ctypes.c_int64 * len(device_ids))(*device_ids)
            rc = lib.axon_start_nrt_profile(ids, len(device_ids))
        else:
            rc = lib.axon_start_nrt_profile(None, 0)
        if rc != 0:
            raise RuntimeError(f"axon_start_nrt_profile rc={rc}")
        try:
            yield
        finally:
            n = lib.axon_stop_nrt_profile(str(output_dir).encode())
            print(f"profile: {n} file(s) -> {output_dir}", file=sys.stderr)

    mod = types.ModuleType("antenv.axon_hooks")
    _h = [_hook]

    mod.set_axon_ntff_profile_hook = lambda h: _h.__setitem__(0, h)
    mod.get_axon_ntff_profile_hook = lambda: _h[0]
    sys.modules["antenv.axon_hooks"] = mod
    import antenv

    antenv.axon_hooks = mod

    import concourse.bass_utils as bu

    bu.upload_artifacts = lambda tmpdir: str(tmpdir)


# revision 32
# speedup vs baseline: 1.0044x; 1.0044x over previous
"""Complex multi-head attention (B=4, S=2048, D=512, H=8) on 8 TRN2 NeuronCores.

Sharding: core c handles batch b = c//2 and head group hg = c%2 (4 heads each).
Weights are head-sliced host-side; each core computes its 4 heads' attention and
a partial output projection; the host sums the two partials per batch.

All complex arithmetic is expanded host-side into real concatenated operands so
that on-device every step is a plain matmul accumulation (bf16 inputs with f32
PSUM accumulation by default, see BF16_MODE):
  - x is passed as XT = [x_b.T.re ; x_b.T.im]  (contraction dim 1024, on SBUF
    partitions in 8 chunks of 128)
  - Q^T/K^T are produced per head in [64 d_re | 64 d_im] x 2048 layout, so the
    Hermitian score Re(conj(q)k) is a single K=128 matmul.
  - softmax is computed without max subtraction (|scores| <= ~18 on this
    distribution, exp stays well inside fp32 range); the denominator comes from
    an all-ones stationary matmul over Vector-engine pair-sums of P^T chunks.
"""

import os

import numpy as np

import concourse.mybir as mybir
import concourse.tile as tile
from concourse import bacc
from concourse.bass import ds, ts
from concourse.bass_utils import run_bass_kernel_spmd

F32 = mybir.dt.float32
F32R = mybir.dt.float32r
BF16 = mybir.dt.bfloat16
# 0: all fp32r; 1: bf16 projection inputs (halves the DMA-bound startup ramp);
# 2: bf16 everywhere (bf16 matmuls avoid the fp32r self-loading weight
# penalty, ~216 vs ~260 ns per matmul). Measured rel err: 6e-4 / 7e-3 / 9e-3
# against a 2e-2 budget.
BF16_MODE = int(os.environ.get("KERNEL_BF16", "2"))
BF16_PROJ = BF16_MODE >= 1
PROJ_DT = BF16 if BF16_PROJ else F32R
ATTN_DT = BF16 if BF16_MODE >= 2 else F32R

B, S, D = 4, 2048, 512
H, Dh = 8, 64
HPC = 4          # heads per core
SCALE = 1.0 / 8.0  # 1/sqrt(Dh)

_NC = None


def _build():
    nc = bacc.Bacc("TRN2", target_bir_lowering=False, debug=False, num_devices=8)

    xt_d = nc.declare_dram_parameter("xt", [128, 8, S], PROJ_DT, isOutput=False)
    wq_d = nc.declare_dram_parameter("wq", [128, 8, HPC, 128], PROJ_DT, isOutput=False)
    wk_d = nc.declare_dram_parameter("wk", [128, 8, HPC, 128], PROJ_DT, isOutput=False)
    wv_d = nc.declare_dram_parameter("wv", [128, 8, 512], PROJ_DT, isOutput=False)
    r_d = nc.declare_dram_parameter("r", [128, HPC, 1024], ATTN_DT, isOutput=False)
    ones_d = nc.declare_dram_parameter("ones", [128, 128], ATTN_DT, isOutput=False)
    out_d = nc.declare_dram_parameter("out", [S, 1024], F32, isOutput=True)

    Exp = mybir.ActivationFunctionType.Exp

    with tile.TileContext(nc) as tc:
        with tc.tile_pool(name="consts", bufs=1) as consts:
            ones = consts.tile([128, 128], ATTN_DT)
            nc.sync.dma_start(out=ones[:, :], in_=ones_d[:, :])

            with tc.tile_pool(name="qkv", bufs=1) as qkv:
                qt = qkv.tile([128, HPC, S], ATTN_DT)   # [d_ri, h, q]
                kt = qkv.tile([128, HPC, S], ATTN_DT)   # [d_ri, h, k]
                v = qkv.tile([128, 16, 512], ATTN_DT)  # [k%128, k//128, h*128+ri*64+d]

                # ---- phase 1: projections ----
                with (
                    tc.tile_pool(name="ph1", bufs=1) as ph1,
                    tc.tile_pool(name="p1ps", bufs=1, space="PSUM") as p1ps,
                ):
                    xt_s = ph1.tile([128, 8, S], PROJ_DT)
                    w_s = ph1.tile([128, 8, HPC, 128], PROJ_DT)  # wq, then wk
                    wv_s = ph1.tile([128, 8, 512], PROJ_DT)
                    for c in range(8):
                        nc.sync.dma_start(
                            out=xt_s[:, c, 0:1024], in_=xt_d[:, c, 0:1024]
                        )
                        nc.sync.dma_start(
                            out=xt_s[:, c, 1024:2048], in_=xt_d[:, c, 1024:2048]
                        )
                        nc.gpsimd.dma_start(out=w_s[:, c, :, :], in_=wq_d[:, c, :, :])
                    nc.gpsimd.dma_start(out=wv_s[:, :, :], in_=wv_d[:, :, :])

                    # c-major waves of 8 concurrent PSUM accumulation chains:
                    # during the initial DMA, every arriving xt chunk feeds 8
                    # matmuls instead of blocking one group-at-a-time.
                    def _qk_wave(dst, heads, group_major=False):
                        groups = [(h, tg) for h in heads for tg in range(4)]
                        if group_major:
                            # all data resident: drain each group right after
                            # its accumulation so copies don't bunch at the end
                            for gi, (h, tg) in enumerate(groups):
                                gt = p1ps.tile(
                                    [128, 512], F32, name=f"acc{gi}", tag=f"acc{gi}"
                                )
                                for c in range(8):
                                    nc.tensor.matmul(
                                        gt[:, :],
                                        lhsT=w_s[:, c, h, :],
                                        rhs=xt_s[:, c, ts(tg, 512)],
                                        start=(c == 0),
                                        stop=(c == 7),
                                    )
                                nc.vector.tensor_copy(
                                    out=dst[:, h, ts(tg, 512)], in_=gt[:, :]
                                )
                            return
                        tiles = [
                            p1ps.tile([128, 512], F32, name=f"acc{gi}", tag=f"acc{gi}")
                            for gi in range(len(groups))
                        ]
                        for c in range(8):
                            for gi, (h, tg) in enumerate(groups):
                                nc.tensor.matmul(
                                    tiles[gi][:, :],
                                    lhsT=w_s[:, c, h, :],
                                    rhs=xt_s[:, c, ts(tg, 512)],
                                    start=(c == 0),
                                    stop=(c == 7),
                                )
                        for gi, (h, tg) in enumerate(groups):
                            nc.vector.tensor_copy(
                                out=dst[:, h, ts(tg, 512)], in_=tiles[gi][:, :]
                            )

                    def _v_wave(tbs):
                        tbs = list(tbs)
                        tiles = [
                            p1ps.tile([128, 512], F32, name=f"acc{gi}", tag=f"acc{gi}")
                            for gi in range(len(list(tbs)))
                        ]
                        for c in range(8):
                            for gi, tb in enumerate(tbs):
                                nc.tensor.matmul(
                                    tiles[gi][:, :],
                                    lhsT=xt_s[:, c, ts(tb, 128)],
                                    rhs=wv_s[:, c, :],
                                    start=(c == 0),
                                    stop=(c == 7),
                                )
                        for gi, tb in enumerate(tbs):
                            nc.scalar.copy(out=v[:, tb, :], in_=tiles[gi][:, :])

                    # Q^T per head: [128 = (64 re | 64 im), 2048 tokens]
                    _qk_wave(qt, (0, 1))
                    _qk_wave(qt, (2, 3))
                    # wk overwrites the wq slot; the WAR wait on Q's last reads
                    # is hidden behind the V projection below
                    for c in range(8):
                        nc.gpsimd.dma_start(out=w_s[:, c, :, :], in_=wk_d[:, c, :, :])
                    # V: [token, 4h x (64 re | 64 im)] in 16 chunks of 128 tokens
                    _v_wave(range(8))
                    _v_wave(range(8, 16))
                    # K^T per head
                    _qk_wave(kt, (0, 1))
                    _qk_wave(kt, (2, 3), group_major=True)

                # ---- phases 2+3 ----
                with tc.tile_pool(name="p23", bufs=1) as p23:
                    ot = p23.tile([128, HPC, S], ATTN_DT)  # [(64 re|64 im), h, q]
                    r_s = p23.tile([128, HPC, 1024], ATTN_DT)
                    nc.gpsimd.dma_start(out=r_s[:, :, :], in_=r_d[:, :, :])

                    # ---- phase 2: attention ----
                    with (
                        tc.tile_pool(name="st", bufs=2, space="PSUM") as stp,
                        tc.tile_pool(name="ov", bufs=1, space="PSUM") as ovp,
                        tc.tile_pool(name="pt", bufs=4) as ptp,
                        tc.tile_pool(name="pair", bufs=3) as prp,
                        tc.tile_pool(name="misc", bufs=1) as miscp,
                    ):
                        def _emit_scores(h, qh, kc):
                            st = stp.tile([128, 1024], F32)
                            for g in range(2):
                                qoff = qh * 1024 + g * 512
                                nc.tensor.matmul(
                                    st[:, ts(g, 512)],
                                    lhsT=kt[:, h, ts(kc, 128)],
                                    rhs=qt[:, h, ds(qoff, 512)],
                                    start=True,
                                    stop=True,
                                )
                            return st

                        iters = [(h, qh) for h in range(HPC) for qh in range(2)]
                        next_st0 = _emit_scores(0, 0, 0)
                        for it, (h, qh) in enumerate(iters):
                            if True:
                                o_ps0 = ovp.tile([128, 512], F32, tag="o0")
                                o_ps1 = ovp.tile([128, 512], F32, tag="o1")
                                o_halves = (o_ps0, o_ps1)
                                d_ps = ovp.tile([128, 1024], F32, tag="d")
                                pts = []
                                pairs = []

                                def _emit_dmm(pi, g, stop=False):
                                    nc.tensor.matmul(
                                        d_ps[:, ts(g, 512)],
                                        lhsT=ones[:, :],
                                        rhs=pairs[pi][:, ts(g, 512)],
                                        start=(pi == 0),
                                        stop=stop,
                                    )

                                sts = {0: next_st0}
                                for kc in range(16):
                                    pt_t = ptp.tile([128, 1024], ATTN_DT)
                                    nc.scalar.activation(
                                        out=pt_t[:, :], in_=sts.pop(kc)[:, :],
                                        func=Exp, scale=SCALE,
                                    )
                                    pts.append(pt_t)
                                    # scores for the NEXT chunk go on the PE
                                    # queue ahead of this chunk's exp-dependent
                                    # matmuls, so the PE computes them while
                                    # the ScalarE runs exp (keeps ACT streaming)
                                    if kc + 1 < 16:
                                        sts[kc + 1] = _emit_scores(h, qh, kc + 1)
                                    elif it + 1 < len(iters):
                                        # first scores of the NEXT (h, qh)
                                        # iteration: gives the PE work during
                                        # the last exp and lets the next exp
                                        # start with no boundary gap
                                        nh, nqh = iters[it + 1]
                                        next_st0 = _emit_scores(nh, nqh, 0)
                                    for g in range(2):
                                        nc.tensor.matmul(
                                            o_halves[g][:, :],
                                            lhsT=v[:, kc, ds(h * 128, 128)],
                                            rhs=pt_t[:, ts(g, 512)],
                                            start=(kc == 0),
                                            stop=(kc == 15),
                                        )
                                    if kc % 2 == 1 and kc < 14:
                                        # denominator: pair-sum P^T chunks on
                                        # the Vector engine, then one all-ones
                                        # matmul per pair (instead of two)
                                        pr = prp.tile([128, 1024], ATTN_DT)
                                        nc.vector.tensor_add(
                                            pr[:, :], pts[kc - 1][:, :], pts[kc][:, :]
                                        )
                                        pairs.append(pr)
                                        # the PREVIOUS pair's denominator
                                        # matmuls: by now its vector add is done
                                        if len(pairs) >= 2:
                                            for g in range(2):
                                                _emit_dmm(len(pairs) - 2, g)
                                # tail: last pair, then the final two chunks go
                                # straight into the denominator (no pair-add on
                                # the critical path after the last exp)
                                for g in range(2):
                                    _emit_dmm(6, g)
                                for kc_t in (14, 15):
                                    for g in range(2):
                                        nc.tensor.matmul(
                                            d_ps[:, ts(g, 512)],
                                            lhsT=ones[:, :],
                                            rhs=pts[kc_t][:, ts(g, 512)],
                                            start=False,
                                            stop=(kc_t == 15),
                                        )
                                recip = miscp.tile([128, 1024], F32, tag="recip")
                                nc.vector.reciprocal_approx_fast(
                                    out=recip[:, :], in_=d_ps[:, :]
                                )
                                for g in range(2):
                                    nc.vector.tensor_mul(
                                        ot[:, h, ds(qh * 1024 + g * 512, 512)],
                                        o_halves[g][:, :],
                                        recip[:, ts(g, 512)],
                                    )

                    # ---- phase 3: output projection (partial over this core's heads) ----
                    with (
                        tc.tile_pool(name="yps", bufs=2, space="PSUM") as yps,
                        tc.tile_pool(name="ysb", bufs=4) as ysb,
                    ):
                        for tb in range(16):
                            y_ps = yps.tile([128, 1024], F32)
                            for g in range(2):
                                for hc in range(HPC):
                                    nc.tensor.matmul(
                                        y_ps[:, ts(g, 512)],
                                        lhsT=ot[:, hc, ts(tb, 128)],
                                        rhs=r_s[:, hc, ts(g, 512)],
                                        start=(hc == 0),
                                        stop=(hc == 3),
                                    )
                            y_s = ysb.tile([128, 1024], F32)
                            if tb % 2 == 0:
                                nc.vector.tensor_copy(out=y_s[:, :], in_=y_ps[:, :])
                                nc.sync.dma_start(
                                    out=out_d[ts(tb, 128), :], in_=y_s[:, :]
                                )
                            else:
                                nc.scalar.copy(out=y_s[:, :], in_=y_ps[:, :])
                                nc.gpsimd.dma_start(
                                    out=out_d[ts(tb, 128), :], in_=y_s[:, :]
                                )

    nc.compile()
    return nc


def _wcat_head(w_h):
    """[64, 512] complex head-slice of a projection weight -> [1024, 128] real
    stationary block: out column j<64 produces re(head feature j), j>=64 im."""
    wr = np.ascontiguousarray(w_h.real).astype(np.float32)
    wi = np.ascontiguousarray(w_h.imag).astype(np.float32)
    top = np.concatenate([wr.T, wi.T], axis=1)     # x_re rows
    bot = np.concatenate([-wi.T, wr.T], axis=1)    # x_im rows
    return np.concatenate([top, bot], axis=0)      # [1024, 128]


def _core_inputs(x, wq, wk, wv, wo, core):
    b, hg = divmod(core, 2)
    heads = [hg * HPC + h for h in range(HPC)]

    xt = np.concatenate(
        [x[b].T.real.astype(np.float32), x[b].T.imag.astype(np.float32)], axis=0
    )  # [1024, 2048]
    xt = np.ascontiguousarray(xt.reshape(8, 128, S).transpose(1, 0, 2))

    def _wqk(w):
        blocks = np.stack(
            [_wcat_head(w[gh * Dh : (gh + 1) * Dh]) for gh in heads]
        )  # [4, 1024, 128]
        return np.ascontiguousarray(
            blocks.reshape(HPC, 8, 128, 128).transpose(2, 1, 0, 3)
        )  # [128, 8, 4, 128]

    wv_cat = np.concatenate(
        [_wcat_head(wv[gh * Dh : (gh + 1) * Dh]) for gh in heads], axis=1
    )  # [1024, 512]
    wv_cat = np.ascontiguousarray(wv_cat.reshape(8, 128, 512).transpose(1, 0, 2))

    r_blocks = []
    for gh in heads:
        wo_h = wo[:, gh * Dh : (gh + 1) * Dh]  # [512, 64] complex
        wor = np.ascontiguousarray(wo_h.real).astype(np.float32)
        woi = np.ascontiguousarray(wo_h.imag).astype(np.float32)
        top = np.concatenate([wor.T, woi.T], axis=1)    # O_re rows -> [64, 1024]
        bot = np.concatenate([-woi.T, wor.T], axis=1)   # O_im rows
        r_blocks.append(np.concatenate([top, bot], axis=0))  # [128, 1024]
    r_cat = np.concatenate(r_blocks, axis=0)  # [512, 1024]
    r_cat = np.ascontiguousarray(r_cat.reshape(HPC, 128, 1024).transpose(1, 0, 2))

    out = {
        "xt": xt,
        "wq": _wqk(wq),
        "wk": _wqk(wk),
        "wv": wv_cat,
        "r": r_cat,
        "ones": np.ones((128, 128), dtype=np.float32),
    }
    if BF16_PROJ:
        import ml_dtypes

        for k in ("xt", "wq", "wk", "wv"):
            out[k] = out[k].astype(ml_dtypes.bfloat16)
    if BF16_MODE >= 2:
        import ml_dtypes

        for k in ("r", "ones"):
            out[k] = out[k].astype(ml_dtypes.bfloat16)
    return out


def kernel(x, wq, wk, wv, wo):
    global _NC
    x = np.asarray(x)
    wq = np.asarray(wq)
    wk = np.asarray(wk)
    wv = np.asarray(wv)
    wo = np.asarray(wo)

    if _NC is None:
        _NC = _build()

    in_maps = [_core_inputs(x, wq, wk, wv, wo, c) for c in range(8)]

    trace = os.environ.get("KERNEL_PROFILE", "0") == "1"
    kwargs = {}
    if trace:
        _install_profile_shim()
        kwargs = {"trace": True}
    res = run_bass_kernel_spmd(_NC, in_maps, core_ids=list(range(8)), **kwargs)
    if trace:
        print(f"HW exec time: {res.exec_time_ns} ns")

    out = np.zeros((B, S, D), dtype=np.complex64)
    for c in range(8):
        b = c // 2
        y = res.results[c]["out"]
        out[b] += y[:, :512] + 1j * y[:, 512:]
    return out


def _install_profile_shim():
    """Register the NTFF profile hook for axon (missing antenv.axon_hooks)."""
    import contextlib
    import ctypes
    import sys
    import types

    try:
        import antenv.axon_hooks  # noqa: F401

        return
    except ImportError:
        pass

    so_path = "/opt/axon/libaxon_pjrt.so"
    lib = ctypes.CDLL(so_path)
    if not hasattr(lib, "axon_start_nrt_profile"):
        return
    lib.axon_start_nrt_profile.argtypes = [
        ctypes.POINTER(ctypes.c_int64),
        ctypes.c_size_t,
    ]
    lib.axon_start_nrt_profile.restype = ctypes.c_int64
    lib.axon_stop_nrt_profile.argtypes = [ctypes.c_char_p]
    lib.axon_stop_nrt_profile.restype = ctypes.c_int64

    @contextlib.contextmanager
    def _hook(output_dir, device_ids):
        import jax

        jax.devices()
        if device_ids:
            ids = (ctypes.c_int64 * len(device_ids))(*device_ids)
            rc = lib.axon_start_nrt_profile(ids, len(device_ids))
        else:
            rc = lib.axon_start_nrt_profile(None, 0)
        if rc != 0:
            raise RuntimeError(f"axon_start_nrt_profile rc={rc}")
        try:
            yield
        finally:
            n = lib.axon_stop_nrt_profile(str(output_dir).encode())
            print(f"profile: {n} file(s) -> {output_dir}", file=sys.stderr)

    mod = types.ModuleType("antenv.axon_hooks")
    _h = [_hook]

    mod.set_axon_ntff_profile_hook = lambda h: _h.__setitem__(0, h)
    mod.get_axon_ntff_profile_hook = lambda: _h[0]
    sys.modules["antenv.axon_hooks"] = mod
    import antenv

    antenv.axon_hooks = mod

    import concourse.bass_utils as bu

    bu.upload_artifacts = lambda tmpdir: str(tmpdir)


# revision 33
# speedup vs baseline: 1.0099x; 1.0055x over previous
"""Complex multi-head attention (B=4, S=2048, D=512, H=8) on 8 TRN2 NeuronCores.

Sharding: core c handles batch b = c//2 and head group hg = c%2 (4 heads each).
Weights are head-sliced host-side; each core computes its 4 heads' attention and
a partial output projection; the host sums the two partials per batch.

All complex arithmetic is expanded host-side into real concatenated operands so
that on-device every step is a plain matmul accumulation (bf16 inputs with f32
PSUM accumulation by default, see BF16_MODE):
  - x is passed as XT = [x_b.T.re ; x_b.T.im]  (contraction dim 1024, on SBUF
    partitions in 8 chunks of 128)
  - Q^T/K^T are produced per head in [64 d_re | 64 d_im] x 2048 layout, so the
    Hermitian score Re(conj(q)k) is a single K=128 matmul.
  - softmax is computed without max subtraction (|scores| <= ~18 on this
    distribution, exp stays well inside fp32 range); the denominator comes from
    an all-ones stationary matmul over Vector-engine pair-sums of P^T chunks.
"""

import os

import numpy as np

import concourse.mybir as mybir
import concourse.tile as tile
from concourse import bacc
from concourse.bass import ds, ts
from concourse.bass_utils import run_bass_kernel_spmd

F32 = mybir.dt.float32
F32R = mybir.dt.float32r
BF16 = mybir.dt.bfloat16
# 0: all fp32r; 1: bf16 projection inputs (halves the DMA-bound startup ramp);
# 2: bf16 everywhere (bf16 matmuls avoid the fp32r self-loading weight
# penalty, ~216 vs ~260 ns per matmul). Measured rel err: 6e-4 / 7e-3 / 9e-3
# against a 2e-2 budget.
BF16_MODE = int(os.environ.get("KERNEL_BF16", "2"))
BF16_PROJ = BF16_MODE >= 1
PROJ_DT = BF16 if BF16_PROJ else F32R
ATTN_DT = BF16 if BF16_MODE >= 2 else F32R

B, S, D = 4, 2048, 512
H, Dh = 8, 64
HPC = 4          # heads per core
SCALE = 1.0 / 8.0  # 1/sqrt(Dh)

_NC = None


def _build():
    nc = bacc.Bacc("TRN2", target_bir_lowering=False, debug=False, num_devices=8)

    xt_d = nc.declare_dram_parameter("xt", [128, 8, S], PROJ_DT, isOutput=False)
    wq_d = nc.declare_dram_parameter("wq", [128, 8, HPC, 128], PROJ_DT, isOutput=False)
    wk_d = nc.declare_dram_parameter("wk", [128, 8, HPC, 128], PROJ_DT, isOutput=False)
    wv_d = nc.declare_dram_parameter("wv", [128, 8, 512], PROJ_DT, isOutput=False)
    r_d = nc.declare_dram_parameter("r", [128, HPC, 1024], ATTN_DT, isOutput=False)
    ones_d = nc.declare_dram_parameter("ones", [128, 128], ATTN_DT, isOutput=False)
    out_d = nc.declare_dram_parameter("out", [S, 1024], F32, isOutput=True)

    Exp = mybir.ActivationFunctionType.Exp

    with tile.TileContext(nc) as tc:
        with tc.tile_pool(name="consts", bufs=1) as consts:
            ones = consts.tile([128, 128], ATTN_DT)
            nc.sync.dma_start(out=ones[:, :], in_=ones_d[:, :])

            with tc.tile_pool(name="qkv", bufs=1) as qkv:
                qt = qkv.tile([128, HPC, S], ATTN_DT)   # [d_ri, h, q]
                kt = qkv.tile([128, HPC, S], ATTN_DT)   # [d_ri, h, k]
                v = qkv.tile([128, 16, 512], ATTN_DT)  # [k%128, k//128, h*128+ri*64+d]

                # ---- phase 1: projections ----
                with (
                    tc.tile_pool(name="ph1", bufs=1) as ph1,
                    tc.tile_pool(name="p1ps", bufs=1, space="PSUM") as p1ps,
                ):
                    xt_s = ph1.tile([128, 8, S], PROJ_DT)
                    w_s = ph1.tile([128, 8, HPC, 128], PROJ_DT)  # wq, then wk
                    wv_s = ph1.tile([128, 8, 512], PROJ_DT)
                    for c in range(8):
                        nc.sync.dma_start(
                            out=xt_s[:, c, 0:1024], in_=xt_d[:, c, 0:1024]
                        )
                        nc.sync.dma_start(
                            out=xt_s[:, c, 1024:2048], in_=xt_d[:, c, 1024:2048]
                        )
                        nc.gpsimd.dma_start(out=w_s[:, c, :, :], in_=wq_d[:, c, :, :])
                    nc.gpsimd.dma_start(out=wv_s[:, :, :], in_=wv_d[:, :, :])

                    # c-major waves of 8 concurrent PSUM accumulation chains:
                    # during the initial DMA, every arriving xt chunk feeds 8
                    # matmuls instead of blocking one group-at-a-time.
                    def _qk_wave(dst, heads, group_major=False):
                        groups = [(h, tg) for h in heads for tg in range(4)]
                        if group_major:
                            # all data resident: drain each group right after
                            # its accumulation so copies don't bunch at the end
                            for gi, (h, tg) in enumerate(groups):
                                gt = p1ps.tile(
                                    [128, 512], F32, name=f"acc{gi}", tag=f"acc{gi}"
                                )
                                for c in range(8):
                                    nc.tensor.matmul(
                                        gt[:, :],
                                        lhsT=w_s[:, c, h, :],
                                        rhs=xt_s[:, c, ts(tg, 512)],
                                        start=(c == 0),
                                        stop=(c == 7),
                                    )
                                nc.vector.tensor_copy(
                                    out=dst[:, h, ts(tg, 512)], in_=gt[:, :]
                                )
                            return
                        tiles = [
                            p1ps.tile([128, 512], F32, name=f"acc{gi}", tag=f"acc{gi}")
                            for gi in range(len(groups))
                        ]
                        for c in range(8):
                            for gi, (h, tg) in enumerate(groups):
                                nc.tensor.matmul(
                                    tiles[gi][:, :],
                                    lhsT=w_s[:, c, h, :],
                                    rhs=xt_s[:, c, ts(tg, 512)],
                                    start=(c == 0),
                                    stop=(c == 7),
                                )
                        for gi, (h, tg) in enumerate(groups):
                            nc.vector.tensor_copy(
                                out=dst[:, h, ts(tg, 512)], in_=tiles[gi][:, :]
                            )

                    def _v_wave(tbs):
                        tbs = list(tbs)
                        tiles = [
                            p1ps.tile([128, 512], F32, name=f"acc{gi}", tag=f"acc{gi}")
                            for gi in range(len(list(tbs)))
                        ]
                        for c in range(8):
                            for gi, tb in enumerate(tbs):
                                nc.tensor.matmul(
                                    tiles[gi][:, :],
                                    lhsT=xt_s[:, c, ts(tb, 128)],
                                    rhs=wv_s[:, c, :],
                                    start=(c == 0),
                                    stop=(c == 7),
                                )
                        for gi, tb in enumerate(tbs):
                            nc.scalar.copy(out=v[:, tb, :], in_=tiles[gi][:, :])

                    # Q^T per head: [128 = (64 re | 64 im), 2048 tokens]
                    _qk_wave(qt, (0, 1))
                    _qk_wave(qt, (2, 3))
                    # wk overwrites the wq slot; the WAR wait on Q's last reads
                    # is hidden behind the V projection below
                    for c in range(8):
                        nc.gpsimd.dma_start(out=w_s[:, c, :, :], in_=wk_d[:, c, :, :])
                    # V: [token, 4h x (64 re | 64 im)] in 16 chunks of 128 tokens
                    _v_wave(range(8))
                    _v_wave(range(8, 16))
                    # K^T per head
                    _qk_wave(kt, (0, 1))
                    _qk_wave(kt, (2, 3), group_major=True)

                # ---- phases 2+3 ----
                with tc.tile_pool(name="p23", bufs=1) as p23:
                    ot = p23.tile([128, HPC, S], ATTN_DT)  # [(64 re|64 im), h, q]
                    r_s = p23.tile([128, HPC, 1024], ATTN_DT)
                    nc.gpsimd.dma_start(out=r_s[:, :, :], in_=r_d[:, :, :])

                    # ---- phase 2: attention ----
                    with (
                        tc.tile_pool(name="st", bufs=2, space="PSUM") as stp,
                        tc.tile_pool(name="ov", bufs=1, space="PSUM") as ovp,
                        tc.tile_pool(name="pt", bufs=4) as ptp,
                        tc.tile_pool(name="pair", bufs=3) as prp,
                        tc.tile_pool(name="misc", bufs=1) as miscp,
                    ):
                        def _emit_scores(h, qh, kc):
                            st = stp.tile([128, 1024], F32)
                            for g in range(2):
                                qoff = qh * 1024 + g * 512
                                nc.tensor.matmul(
                                    st[:, ts(g, 512)],
                                    lhsT=kt[:, h, ts(kc, 128)],
                                    rhs=qt[:, h, ds(qoff, 512)],
                                    start=True,
                                    stop=True,
                                )
                            return st

                        iters = [(h, qh) for h in range(HPC) for qh in range(2)]
                        next_st0 = _emit_scores(0, 0, 0)
                        for it, (h, qh) in enumerate(iters):
                            if True:
                                o_ps0 = ovp.tile([128, 512], F32, tag="o0")
                                o_ps1 = ovp.tile([128, 512], F32, tag="o1")
                                o_halves = (o_ps0, o_ps1)
                                d_ps = ovp.tile([128, 1024], F32, tag="d")
                                pts = []
                                pairs = []

                                def _emit_dmm(pi, g, stop=False):
                                    nc.tensor.matmul(
                                        d_ps[:, ts(g, 512)],
                                        lhsT=ones[:, :],
                                        rhs=pairs[pi][:, ts(g, 512)],
                                        start=(pi == 0),
                                        stop=stop,
                                    )

                                sts = {0: next_st0}
                                for kc in range(16):
                                    pt_t = ptp.tile([128, 1024], ATTN_DT)
                                    nc.scalar.activation(
                                        out=pt_t[:, :], in_=sts.pop(kc)[:, :],
                                        func=Exp, scale=SCALE,
                                    )
                                    pts.append(pt_t)
                                    # scores for the NEXT chunk go on the PE
                                    # queue ahead of this chunk's exp-dependent
                                    # matmuls, so the PE computes them while
                                    # the ScalarE runs exp (keeps ACT streaming)
                                    if kc + 1 < 16:
                                        sts[kc + 1] = _emit_scores(h, qh, kc + 1)
                                    elif it + 1 < len(iters):
                                        # first scores of the NEXT (h, qh)
                                        # iteration: gives the PE work during
                                        # the last exp and lets the next exp
                                        # start with no boundary gap
                                        nh, nqh = iters[it + 1]
                                        next_st0 = _emit_scores(nh, nqh, 0)
                                    for g in range(2):
                                        nc.tensor.matmul(
                                            o_halves[g][:, :],
                                            lhsT=v[:, kc, ds(h * 128, 128)],
                                            rhs=pt_t[:, ts(g, 512)],
                                            start=(kc == 0),
                                            stop=(kc == 15),
                                        )
                                    if kc % 2 == 1:
                                        # denominator: pair-sum P^T chunks on
                                        # the Vector engine, then one all-ones
                                        # matmul per pair (instead of two)
                                        pr = prp.tile([128, 1024], ATTN_DT)
                                        nc.vector.tensor_add(
                                            pr[:, :], pts[kc - 1][:, :], pts[kc][:, :]
                                        )
                                        pairs.append(pr)
                                        # the PREVIOUS pair's denominator
                                        # matmuls: by now its vector add is done
                                        if len(pairs) >= 2:
                                            for g in range(2):
                                                _emit_dmm(len(pairs) - 2, g)
                                for g in range(2):
                                    _emit_dmm(7, g, stop=True)
                                recip = miscp.tile([128, 1024], F32, tag="recip")
                                nc.vector.reciprocal_approx_fast(
                                    out=recip[:, :], in_=d_ps[:, :]
                                )
                                for g in range(2):
                                    nc.vector.tensor_mul(
                                        ot[:, h, ds(qh * 1024 + g * 512, 512)],
                                        o_halves[g][:, :],
                                        recip[:, ts(g, 512)],
                                    )

                    # ---- phase 3: output projection (partial over this core's heads) ----
                    with (
                        tc.tile_pool(name="yps", bufs=2, space="PSUM") as yps,
                        tc.tile_pool(name="ysb", bufs=4) as ysb,
                    ):
                        for tb in range(16):
                            y_ps = yps.tile([128, 1024], F32)
                            for g in range(2):
                                for hc in range(HPC):
                                    nc.tensor.matmul(
                                        y_ps[:, ts(g, 512)],
                                        lhsT=ot[:, hc, ts(tb, 128)],
                                        rhs=r_s[:, hc, ts(g, 512)],
                                        start=(hc == 0),
                                        stop=(hc == 3),
                                    )
                            y_s = ysb.tile([128, 1024], F32)
                            if tb % 2 == 0:
                                nc.vector.tensor_copy(out=y_s[:, :], in_=y_ps[:, :])
                                nc.sync.dma_start(
                                    out=out_d[ts(tb, 128), :], in_=y_s[:, :]
                                )
                            else:
                                nc.scalar.copy(out=y_s[:, :], in_=y_ps[:, :])
                                nc.gpsimd.dma_start(
                                    out=out_d[ts(tb, 128), :], in_=y_s[:, :]
                                )

    nc.compile()
    return nc


def _wcat_head(w_h):
    """[64, 512] complex head-slice of a projection weight -> [1024, 128] real
    stationary block: out column j<64 produces re(head feature j), j>=64 im."""
    wr = np.ascontiguousarray(w_h.real).astype(np.float32)
    wi = np.ascontiguousarray(w_h.imag).astype(np.float32)
    top = np.concatenate([wr.T, wi.T], axis=1)     # x_re rows
    bot = np.concatenate([-wi.T, wr.T], axis=1)    # x_im rows
    return np.concatenate([top, bot], axis=0)      # [1024, 128]


def _core_inputs(x, wq, wk, wv, wo, core):
    b, hg = divmod(core, 2)
    heads = [hg * HPC + h for h in range(HPC)]

    xt = np.concatenate(
        [x[b].T.real.astype(np.float32), x[b].T.imag.astype(np.float32)], axis=0
    )  # [1024, 2048]
    xt = np.ascontiguousarray(xt.reshape(8, 128, S).transpose(1, 0, 2))

    def _wqk(w):
        blocks = np.stack(
            [_wcat_head(w[gh * Dh : (gh + 1) * Dh]) for gh in heads]
        )  # [4, 1024, 128]
        return np.ascontiguousarray(
            blocks.reshape(HPC, 8, 128, 128).transpose(2, 1, 0, 3)
        )  # [128, 8, 4, 128]

    wv_cat = np.concatenate(
        [_wcat_head(wv[gh * Dh : (gh + 1) * Dh]) for gh in heads], axis=1
    )  # [1024, 512]
    wv_cat = np.ascontiguousarray(wv_cat.reshape(8, 128, 512).transpose(1, 0, 2))

    r_blocks = []
    for gh in heads:
        wo_h = wo[:, gh * Dh : (gh + 1) * Dh]  # [512, 64] complex
        wor = np.ascontiguousarray(wo_h.real).astype(np.float32)
        woi = np.ascontiguousarray(wo_h.imag).astype(np.float32)
        top = np.concatenate([wor.T, woi.T], axis=1)    # O_re rows -> [64, 1024]
        bot = np.concatenate([-woi.T, wor.T], axis=1)   # O_im rows
        r_blocks.append(np.concatenate([top, bot], axis=0))  # [128, 1024]
    r_cat = np.concatenate(r_blocks, axis=0)  # [512, 1024]
    r_cat = np.ascontiguousarray(r_cat.reshape(HPC, 128, 1024).transpose(1, 0, 2))

    out = {
        "xt": xt,
        "wq": _wqk(wq),
        "wk": _wqk(wk),
        "wv": wv_cat,
        "r": r_cat,
        "ones": np.ones((128, 128), dtype=np.float32),
    }
    if BF16_PROJ:
        import ml_dtypes

        for k in ("xt", "wq", "wk", "wv"):
            out[k] = out[k].astype(ml_dtypes.bfloat16)
    if BF16_MODE >= 2:
        import ml_dtypes

        for k in ("r", "ones"):
            out[k] = out[k].astype(ml_dtypes.bfloat16)
    return out


def kernel(x, wq, wk, wv, wo):
    global _NC
    x = np.asarray(x)
    wq = np.asarray(wq)
    wk = np.asarray(wk)
    wv = np.asarray(wv)
    wo = np.asarray(wo)

    if _NC is None:
        _NC = _build()

    in_maps = [_core_inputs(x, wq, wk, wv, wo, c) for c in range(8)]

    trace = os.environ.get("KERNEL_PROFILE", "0") == "1"
    kwargs = {}
    if trace:
        _install_profile_shim()
        kwargs = {"trace": True}
    res = run_bass_kernel_spmd(_NC, in_maps, core_ids=list(range(8)), **kwargs)
    if trace:
        print(f"HW exec time: {res.exec_time_ns} ns")

    out = np.zeros((B, S, D), dtype=np.complex64)
    for c in range(8):
        b = c // 2
        y = res.results[c]["out"]
        out[b] += y[:, :512] + 1j * y[:, 512:]
    return out


def _install_profile_shim():
    """Register the NTFF profile hook for axon (missing antenv.axon_hooks)."""
    import contextlib
    import ctypes
    import sys
    import types

    try:
        import antenv.axon_hooks  # noqa: F401

        return
    except ImportError:
        pass

    so_path = "/opt/axon/libaxon_pjrt.so"
    lib = ctypes.CDLL(so_path)
    if not hasattr(lib, "axon_start_nrt_profile"):
        return
    lib.axon_start_nrt_profile.argtypes = [
        ctypes.POINTER(ctypes.c_int64),
        ctypes.c_size_t,
    ]
    lib.axon_start_nrt_profile.restype = ctypes.c_int64
    lib.axon_stop_nrt_profile.argtypes = [ctypes.c_char_p]
    lib.axon_stop_nrt_profile.restype = ctypes.c_int64

    @contextlib.contextmanager
    def _hook(output_dir, device_ids):
        import jax

        jax.devices()
        if device_ids:
            ids = (ctypes.c_int64 * len(device_ids))(*device_ids)
            rc = lib.axon_start_nrt_profile(ids, len(device_ids))
        else:
            rc = lib.axon_start_nrt_profile(None, 0)
        if rc != 0:
            raise RuntimeError(f"axon_start_nrt_profile rc={rc}")
        try:
            yield
        finally:
            n = lib.axon_stop_nrt_profile(str(output_dir).encode())
            print(f"profile: {n} file(s) -> {output_dir}", file=sys.stderr)

    mod = types.ModuleType("antenv.axon_hooks")
    _h = [_hook]

    mod.set_axon_ntff_profile_hook = lambda h: _h.__setitem__(0, h)
    mod.get_axon_ntff_profile_hook = lambda: _h[0]
    sys.modules["antenv.axon_hooks"] = mod
    import antenv

    antenv.axon_hooks = mod

    import concourse.bass_utils as bu

    bu.upload_artifacts = lambda tmpdir: str(tmpdir)
